# revision 20
# baseline (speedup 1.0000x reference)
"""Trainium2 Bass kernel for causal multi-head attention with NeoX RoPE.

Problem: x[2, 2048, 1024], 16 heads x d_head 64, rotary over all 64 dims,
causal softmax, output projection.  Sharding: head-parallel attention -
core c computes heads {2c, 2c+1} for BOTH batches; the per-head attention
outputs z are exchanged with one 8-core AllToAll per q-chunk (core c ends up
with all 16 heads for batch c//4, q-quarter c%4), then every core runs the
output projection with the full W_O for its 128 q rows per chunk.

Per-core dataflow (2 heads, 2 batches):
  xT [b, d, s] (bf16, host-transposed)
  Q/K projections -> qT/kT [head-pair 128, b, s] via PE; RoPE applied as
    q*cos + shuffle(q)*sin' with the rotate-half shuffle done by cross-
    partition copies and the flip sign folded into the sin table.
  V projection -> V [s, b, head, 65] with a ones column (softmax denominator
    rides the attention matmul for free).
  Scores computed transposed: S_T[k, q] = kT.T @ qT per 128-k-tile, exp on
    ScalarE (scale=1/8 folded in), causal mask via GPSIMD affine_select on
    diagonal tiles only.
  AV: zT_aug[65, q] += V_aug.T @ E_T accumulated over k tiles; row 64 is the
  softmax denominator.  Normalize via reciprocal_approx_fast (read straight
  from PSUM) + GPSIMD partition_broadcast + one tensor_tensor multiply.
  z exchange: zsb -> DRAM -> 8-core AllToAll -> z_all [1024, 128q].
  Output projection: out[q, m] = z_all.T @ W_O_full, f16 out via ScalarE.
"""

import numpy as np
import ml_dtypes

S = 2048
D = 1024
NH = 16
DH = 64
SCALE = 8.0
ROT_BASE = 10000.0
N_CORES = 8
QCHUNK = 512     # q chunk (free dim of score matmuls)
NCHUNK = S // QCHUNK
KTILE = 128
BF = ml_dtypes.bfloat16

_BUILT = {}


def _build(with_qk_bias):
    import concourse.bass as bass
    import concourse.tile as tile
    from concourse import bacc, mybir

    f32 = mybir.dt.float32
    bf16 = mybir.dt.bfloat16
    f16 = mybir.dt.float16
    AF = mybir.ActivationFunctionType
    OP = mybir.AluOpType

    nc = bacc.Bacc("TRN2", target_bir_lowering=False, debug=False,
                   num_devices=N_CORES)

    xT = nc.dram_tensor("xT", [NCHUNK, 128, 2, 8, QCHUNK], bf16,
                    kind="ExternalInput").ap()
    wq = nc.dram_tensor("wq", [128, 8, 128], bf16, kind="ExternalInput").ap()
    wk = nc.dram_tensor("wk", [128, 8, 128], bf16, kind="ExternalInput").ap()
    wv = nc.dram_tensor("wv", [128, 8, 128], bf16, kind="ExternalInput").ap()
    # full W_O for all 16 heads: [p, K, m], d = K*128 + p in head-major order
    wo = nc.dram_tensor("wo", [128, 8, D], bf16, kind="ExternalInput").ap()
    cosd = nc.dram_tensor("cosT", [128, S], bf16, kind="ExternalInput").ap()
    sind = nc.dram_tensor("sinTm", [128, S], bf16, kind="ExternalInput").ap()
    if with_qk_bias:
        bqd = nc.dram_tensor("bq", [128, 1], f32, kind="ExternalInput").ap()
        bkd = nc.dram_tensor("bk", [128, 1], f32, kind="ExternalInput").ap()

    out_ext = nc.dram_tensor("out_shard", [NCHUNK, 128, D], f16,
                             kind="ExternalOutput").ap()
    import os as _os
    _DBG = _os.environ.get("KDBG", "0") == "1"
    if _DBG:
        dbg_zs = nc.dram_tensor("dbg_zs", [NCHUNK, 8, 128, 128], bf16,
                                kind="ExternalOutput").ap()
        dbg_zr = nc.dram_tensor("dbg_zr", [NCHUNK, 8, 128, 128], bf16,
                                kind="ExternalOutput").ap()

    with tile.TileContext(nc) as tc:
        with (
            tc.tile_pool(name="consts", bufs=1) as consts,
            tc.tile_pool(name="qk", bufs=1) as qkpool,
            tc.tile_pool(name="vsb", bufs=1) as vpool,
            tc.tile_pool(name="rope", bufs=2) as rope,
            tc.tile_pool(name="xp", bufs=2) as xpool,
            tc.tile_pool(name="epool", bufs=2) as epool,
            tc.tile_pool(name="zpool", bufs=2) as zpool,
            tc.tile_pool(name="den", bufs=2) as den,
            tc.tile_pool(name="zx", bufs=2) as zxpool,
            tc.tile_pool(name="osb", bufs=2) as osbpool,
            tc.tile_pool(name="ps_sc", bufs=2, space="PSUM") as ps_sc,
            tc.tile_pool(name="ps_av", bufs=2, space="PSUM") as ps_av,
            tc.tile_pool(name="ps_pj", bufs=2, space="PSUM") as ps_pj,
            tc.tile_pool(name="dram", bufs=1, space="DRAM") as dram,
        ):
            # ---- input DMAs: few big transfers, priority-ordered ----
            wq_sb = consts.tile([128, 8, 128], bf16, tag="wq")
            nc.sync.dma_start(out=wq_sb, in_=wq)
            wk_sb = consts.tile([128, 8, 128], bf16, tag="wk")
            nc.scalar.dma_start(out=wk_sb, in_=wk)
            wv_sb = consts.tile([128, 8, 128], bf16, tag="wv")
            nc.gpsimd.dma_start(out=wv_sb, in_=wv)
            wo_sb = consts.tile([128, 8, D], bf16, tag="wo")
            nc.gpsimd.dma_start(out=wo_sb, in_=wo)
            if with_qk_bias:
                bq_sb = consts.tile([128, 1], f32, tag="bq")
                nc.sync.dma_start(out=bq_sb, in_=bqd)
                bk_sb = consts.tile([128, 1], f32, tag="bk")
                nc.sync.dma_start(out=bk_sb, in_=bkd)

            # preload the exp table set so the first real exp is fast
            warm = consts.tile([128, 8], f32, tag="warm")
            nc.vector.memset(warm, 0.0)
            nc.scalar.activation(out=warm, in_=warm, func=AF.Exp, scale=1.0)

            # Persistent rotated Q/K: [128 (= head pair), batch, s]
            Qr = qkpool.tile([128, 2, S], bf16, tag="Qr")
            Kr = qkpool.tile([128, 2, S], bf16, tag="Kr")
            # V with ones column: [s-part, s-tile, batch, head, 65]
            Vs = vpool.tile([128, 16, 2, 2, 65], bf16, tag="Vs")
            nc.vector.memset(Vs[:, :, :, :, 64:65], 1.0)

            # DRAM bounce buffers for the per-chunk z AllToAll.
            # layout [8 dest, 128 p, 128 q']: dest-slab (b, qq) is the
            # collective's split dim.
            zsend = [dram.tile([8, 128, 128], bf16, tag=f"zs{j}",
                               name=f"zsend{j}") for j in range(NCHUNK)]
            zrecv = [dram.tile([8, 128, 128], bf16, tag=f"zr{j}",
                               name=f"zrecv{j}") for j in range(NCHUNK)]

            # x chunks stream through a 2-deep ring; chunk c is consumed
            # only by proj_chunk(c)/proj_v(c) at stage (c, 0).
            xc_tiles = {}

            def fetch_x(c, eng):
                if c >= NCHUNK or c in xc_tiles:
                    return
                xc = xpool.tile([128, 2, 8, QCHUNK], bf16, tag="xT",
                                name=f"xc{c}")
                eng.dma_start(out=xc, in_=xT[c])
                xc_tiles[c] = xc

            # chunk 0 split across both HWDGE queues for fastest arrival
            xc0 = xpool.tile([128, 2, 8, QCHUNK], bf16, tag="xT", name="xc0")
            nc.sync.dma_start(out=xc0[:, 0, 0:4], in_=xT[0, :, 0, 0:4])
            nc.scalar.dma_start(out=xc0[:, 0, 4:8], in_=xT[0, :, 0, 4:8])
            nc.sync.dma_start(out=xc0[:, 1, 0:4], in_=xT[0, :, 1, 0:4])
            nc.scalar.dma_start(out=xc0[:, 1, 4:8], in_=xT[0, :, 1, 4:8])
            xc_tiles[0] = xc0
            cos_sb = consts.tile([128, S], bf16, tag="cos")
            nc.scalar.dma_start(out=cos_sb, in_=cosd)
            sin_sb = consts.tile([128, S], bf16, tag="sin")
            nc.scalar.dma_start(out=sin_sb, in_=sind)
            fetch_x(1, nc.sync)

            # ---- per-chunk projections (interleaved with attention) ----
            def proj_chunk_items(c):
                xc = xc_tiles[c]
                cs = slice(c * QCHUNK, (c + 1) * QCHUNK)
                items = []
                for b in range(2):
                    for (wsb, bias_sb, dst) in (
                        (wq_sb, "bq", Qr), (wk_sb, "bk", Kr)):
                        def emit(b=b, wsb=wsb, bias_sb=bias_sb, dst=dst):
                            pt = ps_pj.tile([128, QCHUNK], f32, tag="pj")
                            for kt in range(8):
                                nc.tensor.matmul(
                                    out=pt, lhsT=wsb[:, kt, :],
                                    rhs=xc[:, b, kt, :],
                                    start=(kt == 0), stop=(kt == 7))
                            if with_qk_bias:
                                bsb = bq_sb if bias_sb == "bq" else bk_sb
                                nc.vector.tensor_scalar_add(
                                    out=pt, in0=pt, scalar1=bsb[:, 0:1])
                            q_sb = rope.tile([128, QCHUNK], bf16, tag="ropeA")
                            nc.scalar.copy(out=q_sb, in_=pt)
                            qf = rope.tile([128, QCHUNK], bf16, tag="ropeB")
                            for blk in range(4):
                                srcp = (blk ^ 1) * 32
                                nc.vector.tensor_copy(
                                    out=qf[blk * 32:blk * 32 + 32, :],
                                    in_=q_sb[srcp:srcp + 32, :])
                            # q_rot = q*cos + flip(q)*sin' (sign in sin')
                            qc = rope.tile([128, QCHUNK], bf16, tag="ropeC")
                            nc.vector.tensor_tensor(
                                out=qc, in0=q_sb, in1=cos_sb[:, cs],
                                op=OP.mult)
                            qs = rope.tile([128, QCHUNK], bf16, tag="ropeD")
                            nc.vector.tensor_tensor(
                                out=qs, in0=qf, in1=sin_sb[:, cs],
                                op=OP.mult)
                            nc.vector.tensor_tensor(
                                out=dst[:, b, cs], in0=qc, in1=qs, op=OP.add)
                        items.append(emit)
                return items

            def proj_v_items(c):
                xc = xc_tiles[c]
                items = []
                for b in range(2):
                    for lst in range(4):
                        def emit(b=b, lst=lst):
                            st = 4 * c + lst
                            pt = ps_pj.tile([128, 128], f32, tag="pj")
                            for kt in range(8):
                                nc.tensor.matmul(
                                    out=pt,
                                    lhsT=xc[:, b, kt,
                                            lst * 128:(lst + 1) * 128],
                                    rhs=wv_sb[:, kt, :],
                                    start=(kt == 0), stop=(kt == 7))
                            nc.vector.tensor_copy(
                                out=Vs[:, st, b, :, 0:64],
                                in_=pt.rearrange("p (h d) -> p h d", h=2))
                        items.append(emit)
                return items

            # ---- attention ----
            E_tiles = {}
            zsb_tiles = {}

            def scores_tile(j, b, t):
                # heads 2c (partitions 0:64) and 2c+1 (64:128) run as
                # concurrent PE row-groups.
                E = E_tiles[(j, b)]
                q0 = max(0, 128 * (t - 4 * j))
                qs2 = slice(j * QCHUNK + q0, (j + 1) * QCHUNK)
                sc = ps_sc.tile([128, 2, QCHUNK], f32, tag="sc")
                for hh in range(2):
                    hs = slice(64 * hh, 64 * hh + 64)
                    nc.tensor.matmul(
                        out=sc[:, hh, q0:],
                        lhsT=Kr[hs, b, t * 128:(t + 1) * 128],
                        rhs=Qr[hs, b, qs2], start=True, stop=True)
                nc.scalar.activation(
                    out=E[:, t, :, q0:], in_=sc[:, :, q0:],
                    func=AF.Exp, scale=1.0 / SCALE)

            def mask(j, b):
                # per diagonal tile dt, only the written span [q0:] is read
                # downstream; with v = q' - q0 the causal test is v >= p.
                E = E_tiles[(j, b)]
                for dt in range(4):
                    q0 = 128 * dt
                    nc.gpsimd.affine_select(
                        out=E[:, 4 * j + dt, :, q0:],
                        in_=E[:, 4 * j + dt, :, q0:],
                        pattern=[[0, 2], [1, QCHUNK - q0]], base=0,
                        channel_multiplier=-1,
                        compare_op=OP.is_ge, fill=0.0)

            def av_epilogue(j, b, hh, z, zsb):
                hs = slice(64 * hh, 64 * hh + 64)
                # one PSUM->SBUF copy frees the accumulation bank fast;
                # the normalize chain then runs entirely from SBUF.
                zc = den.tile([65, QCHUNK], f32, tag="zc")
                nc.vector.tensor_copy(out=zc, in_=z)
                d0 = den.tile([1, QCHUNK], f32, tag="d0")
                nc.vector.tensor_copy(out=d0, in_=zc[64:65, :])
                r0 = den.tile([1, QCHUNK], f32, tag="r0")
                nc.vector.reciprocal_approx_fast(out=r0, in_=d0)
                rb = den.tile([64, QCHUNK], f32, tag="rb")
                nc.gpsimd.partition_broadcast(out_ap=rb, in_ap=r0)
                nc.vector.tensor_tensor(
                    out=zsb[hs, b, :], in0=zc[0:64, :], in1=rb,
                    op=OP.mult)

            def av_items(j, b):
                """Per-matmul emitters for the AV accumulation of (j, b);
                interleaved into the next stage's scores stream so the PE
                always has a ready matmul while exp drains sc banks."""
                nkt = 4 * j + 4
                E = E_tiles.pop((j, b))
                if b == 0:
                    zsb_tiles[j] = zpool.tile([128, 2, QCHUNK], bf16,
                                              tag="zsb", name="zsb")
                zsb = zsb_tiles[j]
                state = {}
                items = []

                def mk(hh, t):
                    def emit():
                        q0 = max(0, 128 * (t - 4 * j))
                        if t == 0:
                            state[hh] = ps_av.tile([65, QCHUNK], f32,
                                                   tag="av", name="z")
                        z = state[hh]
                        nc.tensor.matmul(
                            out=z[:, q0:], lhsT=Vs[:, t, b, hh, :],
                            rhs=E[:, t, hh, q0:],
                            start=(t == 0), stop=(t == nkt - 1))
                        if t == nkt - 1:
                            av_epilogue(j, b, hh, z, zsb)
                    return emit

                for hh in range(2):
                    for t in range(nkt):
                        items.append(mk(hh, t))
                return items

            def z_export(j):
                # zsb [p, b, q] -> zsend [dest=(b,qq), p, q']; q = qq*128+q'
                zsb = zsb_tiles.pop(j)
                src = zsb.rearrange("p b (r q) -> p (b r) q", r=4)
                dst = zsend[j].rearrange("c p q -> p c q")
                nc.sync.dma_start(out=dst, in_=src)
                nc.gpsimd.collective_compute(
                    "AllToAll", mybir.AluOpType.bypass,
                    replica_groups=[list(range(N_CORES))],
                    ins=[zsend[j].opt()],
                    outs=[zrecv[j].opt()])

            def outproj(j):
                # zrecv [src s, p, q'] -> zx_sb [p, K=s, q']
                zx_sb = zxpool.tile([128, 8, 128], bf16, tag="zx")
                nc.sync.dma_start(
                    out=zx_sb,
                    in_=zrecv[j].rearrange("s p q -> p s q"))
                o_sb = osbpool.tile([128, D], f16, tag="osb")
                po0 = ps_pj.tile([128, 512], f32, tag="pj")
                po1 = ps_pj.tile([128, 512], f32, tag="pj")
                for kt in range(8):
                    for mc, po in ((0, po0), (1, po1)):
                        nc.tensor.matmul(
                            out=po, lhsT=zx_sb[:, kt, :],
                            rhs=wo_sb[:, kt, mc * 512:(mc + 1) * 512],
                            start=(kt == 0), stop=(kt == 7))
                for mc, po in ((0, po0), (1, po1)):
                    nc.scalar.copy(
                        out=o_sb[:, mc * 512:(mc + 1) * 512], in_=po)
                nc.sync.dma_start(out=out_ext[j], in_=o_sb)

            stages = [(j, b) for j in range(NCHUNK) for b in range(2)]

            def weave(primary, filler):
                """Emit all of `primary`, spreading `filler` between them."""
                n = max(1, len(primary))
                ratio = (len(filler) + n - 1) // n if filler else 0
                fi = 0
                for p in primary:
                    p()
                    for _ in range(ratio):
                        if fi < len(filler):
                            filler[fi]()
                            fi += 1
                while fi < len(filler):
                    filler[fi]()
                    fi += 1

            for idx, (j, b) in enumerate(stages):
                nkt = 4 * j + 4
                E_tiles[(j, b)] = epool.tile([128, 16, 2, QCHUNK], bf16,
                                             tag="E", name="E")
                sc_items = [(lambda t=t: scores_tile(j, b, t))
                            for t in range(nkt)]
                if b == 0:
                    fetch_x(j + 1, nc.sync)
                    # av(j-1,1) weaves into proj so its normalize lands on
                    # the DVE queue ahead of this chunk's rope ops.
                    av_prev = av_items(j - 1, 1) if idx >= 1 else []
                    weave(proj_chunk_items(j), av_prev)
                    if idx >= 1:
                        z_export(j - 1)
                    weave(sc_items, proj_v_items(j))
                    if idx >= 4:
                        outproj(j - 2)
                else:
                    weave(sc_items, av_items(j, 0))
                mask(j, b)
            for it in av_items(3, 1):
                it()
            z_export(3)
            outproj(2)
            outproj(3)

    nc.compile()
    return nc


def _get_built(with_qk_bias):
    key = bool(with_qk_bias)
    if key not in _BUILT:
        _BUILT[key] = _build(key)
    return _BUILT[key]


def _rope_tables():
    pos = np.arange(S, dtype=np.float64)
    dim = np.arange(DH // 2, dtype=np.float64)
    freq = ROT_BASE ** (dim / (DH / 2))
    freq = np.concatenate([freq, freq])                # [64]
    ang = pos[None, :] / freq[:, None]                 # [64, S]
    cos = np.cos(ang)
    sin = np.sin(ang)
    # sign of the rotate-half term folded into sin': rows 0..31 get -sin
    sinm = sin.copy()
    sinm[:DH // 2] *= -1.0
    cosT = np.tile(cos, (2, 1)).astype(BF)             # [128, S]
    sinT = np.tile(sinm, (2, 1)).astype(BF)
    return cosT, sinT


def kernel(x, W_Q, b_Q, W_K, b_K, W_V, b_V, W_O, b_O):
    from concourse.bass_utils import run_bass_kernel_spmd

    x = np.asarray(x)
    W_Q, W_K, W_V, W_O = (np.asarray(a) for a in (W_Q, W_K, W_V, W_O))
    b_Q, b_K, b_V, b_O = (np.asarray(a) for a in (b_Q, b_K, b_V, b_O))
    with_qk_bias = bool(np.any(b_Q) or np.any(b_K))
    nc = _get_built(with_qk_bias)

    cosT, sinT = _rope_tables()

    def wtile(w):            # [1024, C] -> [128, 8, C]
        c = w.shape[1]
        return np.ascontiguousarray(
            w.reshape(8, 128, c).transpose(1, 0, 2)).astype(BF)

    # full output projection, head-major rows: [1024, 1024] -> [128, 8, 1024]
    wo_full = np.ascontiguousarray(
        W_O.reshape(NH * DH, D).reshape(8, 128, D).transpose(1, 0, 2)
    ).astype(BF)

    # chunk-major x so each chunk's DMA is one contiguous 2MB read:
    # [NCHUNK, 128, 2, 8, QCHUNK]
    xt = np.stack([x[b].T.reshape(8, 128, S).transpose(1, 0, 2)
                   for b in range(2)], axis=1)          # [128, 2, 8, S]
    xT_host = np.ascontiguousarray(
        xt.reshape(128, 2, 8, NCHUNK, QCHUNK).transpose(3, 0, 1, 2, 4)
    ).astype(BF)

    in_maps = []
    for core in range(N_CORES):
        h0 = 2 * core
        wq_h = wtile(np.concatenate([W_Q[h0], W_Q[h0 + 1]], axis=1))
        wk_h = wtile(np.concatenate([W_K[h0], W_K[h0 + 1]], axis=1))
        wv_h = wtile(np.concatenate([W_V[h0], W_V[h0 + 1]], axis=1))
        m = {
            "xT": xT_host, "wq": wq_h, "wk": wk_h, "wv": wv_h, "wo": wo_full,
            "cosT": cosT, "sinTm": sinT,
        }
        if with_qk_bias:
            m["bq"] = np.concatenate(
                [b_Q[h0], b_Q[h0 + 1]]).astype(np.float32).reshape(128, 1)
            m["bk"] = np.concatenate(
                [b_K[h0], b_K[h0 + 1]]).astype(np.float32).reshape(128, 1)
        in_maps.append(m)

    global _last_in_maps
    _last_in_maps = in_maps
    res = run_bass_kernel_spmd(nc, in_maps, list(range(N_CORES)))

    out = np.empty((2, S, D), dtype=np.float32)
    for core in range(N_CORES):
        b, r = divmod(core, 4)
        shard = res.results[core]["out_shard"].astype(np.float32)
        for j in range(NCHUNK):
            out[b, QCHUNK * j + 128 * r: QCHUNK * j + 128 * (r + 1), :] = \
                shard[j]

    # b_V shifts z by exactly b_V (softmax rows sum to 1); fold with b_O.
    corr = b_O.astype(np.float64).copy()
    if np.any(b_V):
        corr = corr + np.einsum("hd,hdm->m", b_V.astype(np.float64),
                                W_O.astype(np.float64))
    if np.any(corr):
        out = out + corr.astype(np.float32)
    return out


# revision 21
# speedup vs baseline: 1.0695x; 1.0695x over previous
"""Trainium2 Bass kernel for causal multi-head attention with NeoX RoPE.

Problem: x[2, 2048, 1024], 16 heads x d_head 64, rotary over all 64 dims,
causal softmax, output projection.  Sharding: head-parallel attention -
core c computes heads {2c, 2c+1} for BOTH batches; the per-head attention
outputs z are exchanged with one 8-core AllToAll per q-chunk (core c ends up
with all 16 heads for batch c//4, q-quarter c%4), then every core runs the
output projection with the full W_O for its 128 q rows per chunk.

Per-core dataflow (2 heads, 2 batches):
  xT [b, d, s] (bf16, host-transposed)
  Q/K projections -> qT/kT [head-pair 128, b, s] via PE; RoPE applied as
    q*cos + shuffle(q)*sin' with the rotate-half shuffle done by cross-
    partition copies and the flip sign folded into the sin table.
  V projection -> V [s, b, head, 65] with a ones column (softmax denominator
    rides the attention matmul for free).
  Scores computed transposed: S_T[k, q] = kT.T @ qT per 128-k-tile, exp on
    ScalarE (scale=1/8 folded in), causal mask via GPSIMD affine_select on
    diagonal tiles only.
  AV: zT_aug[65, q] += V_aug.T @ E_T accumulated over k tiles; row 64 is the
  softmax denominator.  Normalize via reciprocal_approx_fast (read straight
  from PSUM) + GPSIMD partition_broadcast + one tensor_tensor multiply.
  z exchange: zsb -> DRAM -> 8-core AllToAll -> z_all [1024, 128q].
  Output projection: out[q, m] = z_all.T @ W_O_full, f16 out via ScalarE.
"""

import numpy as np
import ml_dtypes

S = 2048
D = 1024
NH = 16
DH = 64
SCALE = 8.0
ROT_BASE = 10000.0
N_CORES = 8
QCHUNK = 512     # q chunk (free dim of score matmuls)
NCHUNK = S // QCHUNK
KTILE = 128
BF = ml_dtypes.bfloat16

_BUILT = {}


def _build(with_qk_bias):
    import concourse.bass as bass
    import concourse.tile as tile
    from concourse import bacc, mybir

    f32 = mybir.dt.float32
    bf16 = mybir.dt.bfloat16
    f16 = mybir.dt.float16
    AF = mybir.ActivationFunctionType
    OP = mybir.AluOpType

    nc = bacc.Bacc("TRN2", target_bir_lowering=False, debug=False,
                   num_devices=N_CORES)

    xT = nc.dram_tensor("xT", [NCHUNK, 128, 2, 8, QCHUNK], bf16,
                    kind="ExternalInput").ap()
    wq = nc.dram_tensor("wq", [128, 8, 128], bf16, kind="ExternalInput").ap()
    wk = nc.dram_tensor("wk", [128, 8, 128], bf16, kind="ExternalInput").ap()
    wv = nc.dram_tensor("wv", [128, 8, 128], bf16, kind="ExternalInput").ap()
    # full W_O for all 16 heads: [p, K, m], d = K*128 + p in head-major order
    wo = nc.dram_tensor("wo", [128, 8, D], bf16, kind="ExternalInput").ap()
    cosd = nc.dram_tensor("cosT", [128, S], bf16, kind="ExternalInput").ap()
    sind = nc.dram_tensor("sinTm", [128, S], bf16, kind="ExternalInput").ap()
    if with_qk_bias:
        bqd = nc.dram_tensor("bq", [128, 1], f32, kind="ExternalInput").ap()
        bkd = nc.dram_tensor("bk", [128, 1], f32, kind="ExternalInput").ap()

    out_ext = nc.dram_tensor("out_shard", [NCHUNK, 128, D], f16,
                             kind="ExternalOutput").ap()
    import os as _os
    _DBG = _os.environ.get("KDBG", "0") == "1"
    if _DBG:
        dbg_zs = nc.dram_tensor("dbg_zs", [NCHUNK, 8, 128, 128], bf16,
                                kind="ExternalOutput").ap()
        dbg_zr = nc.dram_tensor("dbg_zr", [NCHUNK, 8, 128, 128], bf16,
                                kind="ExternalOutput").ap()

    with tile.TileContext(nc) as tc:
        with (
            tc.tile_pool(name="consts", bufs=1) as consts,
            tc.tile_pool(name="qk", bufs=1) as qkpool,
            tc.tile_pool(name="vsb", bufs=1) as vpool,
            tc.tile_pool(name="rope", bufs=2) as rope,
            tc.tile_pool(name="xp", bufs=2) as xpool,
            tc.tile_pool(name="epool", bufs=2) as epool,
            tc.tile_pool(name="zpool", bufs=2) as zpool,
            tc.tile_pool(name="den", bufs=2) as den,
            tc.tile_pool(name="zx", bufs=2) as zxpool,
            tc.tile_pool(name="osb", bufs=2) as osbpool,
            tc.tile_pool(name="ps_sc", bufs=2, space="PSUM") as ps_sc,
            tc.tile_pool(name="ps_av", bufs=2, space="PSUM") as ps_av,
            tc.tile_pool(name="ps_pj", bufs=2, space="PSUM") as ps_pj,
            tc.tile_pool(name="dram", bufs=1, space="DRAM") as dram,
        ):
            # ---- input DMAs: few big transfers, priority-ordered ----
            wq_sb = consts.tile([128, 8, 128], bf16, tag="wq")
            nc.sync.dma_start(out=wq_sb, in_=wq)
            wk_sb = consts.tile([128, 8, 128], bf16, tag="wk")
            nc.scalar.dma_start(out=wk_sb, in_=wk)
            wv_sb = consts.tile([128, 8, 128], bf16, tag="wv")
            nc.gpsimd.dma_start(out=wv_sb, in_=wv)
            wo_sb = consts.tile([128, 8, D], bf16, tag="wo")
            nc.gpsimd.dma_start(out=wo_sb, in_=wo)
            if with_qk_bias:
                bq_sb = consts.tile([128, 1], f32, tag="bq")
                nc.sync.dma_start(out=bq_sb, in_=bqd)
                bk_sb = consts.tile([128, 1], f32, tag="bk")
                nc.sync.dma_start(out=bk_sb, in_=bkd)

            # preload the exp table set so the first real exp is fast
            warm = consts.tile([128, 8], f32, tag="warm")
            nc.vector.memset(warm, 0.0)
            nc.scalar.activation(out=warm, in_=warm, func=AF.Exp, scale=1.0)

            # Persistent rotated Q/K: [128 (= head pair), batch, s]
            Qr = qkpool.tile([128, 2, S], bf16, tag="Qr")
            Kr = qkpool.tile([128, 2, S], bf16, tag="Kr")
            # V with ones column: [s-part, s-tile, batch, head, 65]
            Vs = vpool.tile([128, 16, 2, 2, 65], bf16, tag="Vs")
            nc.vector.memset(Vs[:, :, :, :, 64:65], 1.0)

            # DRAM bounce buffers for the per-chunk z AllToAll.
            # layout [8 dest, 128 p, 128 q']: dest-slab (b, qq) is the
            # collective's split dim.
            zsend = [dram.tile([8, 128, 128], bf16, tag=f"zs{j}",
                               name=f"zsend{j}") for j in range(NCHUNK)]
            zrecv = [dram.tile([8, 128, 128], bf16, tag=f"zr{j}",
                               name=f"zrecv{j}") for j in range(NCHUNK)]

            # x chunks stream through a 2-deep ring; chunk c is consumed
            # only by proj_chunk(c)/proj_v(c) at stage (c, 0).
            xc_tiles = {}

            def fetch_x(c, eng):
                if c >= NCHUNK or c in xc_tiles:
                    return
                xc = xpool.tile([128, 2, 8, QCHUNK], bf16, tag="xT",
                                name=f"xc{c}")
                eng.dma_start(out=xc, in_=xT[c])
                xc_tiles[c] = xc

            # chunk 0 split across both HWDGE queues for fastest arrival
            xc0 = xpool.tile([128, 2, 8, QCHUNK], bf16, tag="xT", name="xc0")
            nc.sync.dma_start(out=xc0[:, 0, 0:4], in_=xT[0, :, 0, 0:4])
            nc.scalar.dma_start(out=xc0[:, 0, 4:8], in_=xT[0, :, 0, 4:8])
            nc.sync.dma_start(out=xc0[:, 1, 0:4], in_=xT[0, :, 1, 0:4])
            nc.scalar.dma_start(out=xc0[:, 1, 4:8], in_=xT[0, :, 1, 4:8])
            xc_tiles[0] = xc0
            cos_sb = consts.tile([128, S], bf16, tag="cos")
            nc.scalar.dma_start(out=cos_sb, in_=cosd)
            sin_sb = consts.tile([128, S], bf16, tag="sin")
            nc.scalar.dma_start(out=sin_sb, in_=sind)
            fetch_x(1, nc.sync)

            # ---- per-chunk projections (interleaved with attention) ----
            def proj_chunk_items(c):
                xc = xc_tiles[c]
                cs = slice(c * QCHUNK, (c + 1) * QCHUNK)
                items = []
                for b in range(2):
                    for (wsb, bias_sb, dst) in (
                        (wq_sb, "bq", Qr), (wk_sb, "bk", Kr)):
                        def emit(b=b, wsb=wsb, bias_sb=bias_sb, dst=dst):
                            pt = ps_pj.tile([128, QCHUNK], f32, tag="pj")
                            for kt in range(8):
                                nc.tensor.matmul(
                                    out=pt, lhsT=wsb[:, kt, :],
                                    rhs=xc[:, b, kt, :],
                                    start=(kt == 0), stop=(kt == 7))
                            if with_qk_bias:
                                bsb = bq_sb if bias_sb == "bq" else bk_sb
                                nc.vector.tensor_scalar_add(
                                    out=pt, in0=pt, scalar1=bsb[:, 0:1])
                            q_sb = rope.tile([128, QCHUNK], bf16, tag="ropeA")
                            nc.vector.tensor_copy(out=q_sb, in_=pt)
                            qf = rope.tile([128, QCHUNK], bf16, tag="ropeB")
                            for blk in range(4):
                                srcp = (blk ^ 1) * 32
                                nc.vector.tensor_copy(
                                    out=qf[blk * 32:blk * 32 + 32, :],
                                    in_=q_sb[srcp:srcp + 32, :])
                            # q_rot = q*cos + flip(q)*sin' (sign in sin')
                            qc = rope.tile([128, QCHUNK], bf16, tag="ropeC")
                            nc.vector.tensor_tensor(
                                out=qc, in0=q_sb, in1=cos_sb[:, cs],
                                op=OP.mult)
                            qs = rope.tile([128, QCHUNK], bf16, tag="ropeD")
                            nc.vector.tensor_tensor(
                                out=qs, in0=qf, in1=sin_sb[:, cs],
                                op=OP.mult)
                            nc.vector.tensor_tensor(
                                out=dst[:, b, cs], in0=qc, in1=qs, op=OP.add)
                        items.append(emit)
                return items

            def proj_v_items(c):
                xc = xc_tiles[c]
                items = []
                for b in range(2):
                    for lst in range(4):
                        def emit(b=b, lst=lst):
                            st = 4 * c + lst
                            pt = ps_pj.tile([128, 128], f32, tag="pj")
                            for kt in range(8):
                                nc.tensor.matmul(
                                    out=pt,
                                    lhsT=xc[:, b, kt,
                                            lst * 128:(lst + 1) * 128],
                                    rhs=wv_sb[:, kt, :],
                                    start=(kt == 0), stop=(kt == 7))
                            nc.vector.tensor_copy(
                                out=Vs[:, st, b, :, 0:64],
                                in_=pt.rearrange("p (h d) -> p h d", h=2))
                        items.append(emit)
                return items

            # ---- attention ----
            E_tiles = {}
            zsb_tiles = {}

            def scores_tile(j, b, t):
                # heads 2c (partitions 0:64) and 2c+1 (64:128) run as
                # concurrent PE row-groups.
                E = E_tiles[(j, b)]
                q0 = max(0, 128 * (t - 4 * j))
                qs2 = slice(j * QCHUNK + q0, (j + 1) * QCHUNK)
                sc = ps_sc.tile([128, 2, QCHUNK], f32, tag="sc")
                for hh in range(2):
                    hs = slice(64 * hh, 64 * hh + 64)
                    nc.tensor.matmul(
                        out=sc[:, hh, q0:],
                        lhsT=Kr[hs, b, t * 128:(t + 1) * 128],
                        rhs=Qr[hs, b, qs2], start=True, stop=True)
                nc.scalar.activation(
                    out=E[:, t, :, q0:], in_=sc[:, :, q0:],
                    func=AF.Exp, scale=1.0 / SCALE)

            def mask(j, b):
                # per diagonal tile dt, only the written span [q0:] is read
                # downstream; with v = q' - q0 the causal test is v >= p.
                E = E_tiles[(j, b)]
                for dt in range(4):
                    q0 = 128 * dt
                    nc.gpsimd.affine_select(
                        out=E[:, 4 * j + dt, :, q0:],
                        in_=E[:, 4 * j + dt, :, q0:],
                        pattern=[[0, 2], [1, QCHUNK - q0]], base=0,
                        channel_multiplier=-1,
                        compare_op=OP.is_ge, fill=0.0)

            def av_epilogue(j, b, hh, z, zsb):
                hs = slice(64 * hh, 64 * hh + 64)
                # one PSUM->SBUF copy frees the accumulation bank fast;
                # the normalize chain then runs entirely from SBUF.
                zc = den.tile([65, QCHUNK], f32, tag="zc")
                nc.vector.tensor_copy(out=zc, in_=z)
                d0 = den.tile([1, QCHUNK], f32, tag="d0")
                nc.vector.tensor_copy(out=d0, in_=zc[64:65, :])
                r0 = den.tile([1, QCHUNK], f32, tag="r0")
                nc.vector.reciprocal_approx_fast(out=r0, in_=d0)
                rb = den.tile([64, QCHUNK], f32, tag="rb")
                nc.gpsimd.partition_broadcast(out_ap=rb, in_ap=r0)
                nc.vector.tensor_tensor(
                    out=zsb[hs, b, :], in0=zc[0:64, :], in1=rb,
                    op=OP.mult)

            def av_items(j, b):
                """Per-matmul emitters for the AV accumulation of (j, b);
                interleaved into the next stage's scores stream so the PE
                always has a ready matmul while exp drains sc banks."""
                nkt = 4 * j + 4
                E = E_tiles.pop((j, b))
                if b == 0:
                    zsb_tiles[j] = zpool.tile([128, 2, QCHUNK], bf16,
                                              tag="zsb", name="zsb")
                zsb = zsb_tiles[j]
                state = {}
                items = []

                def mk(hh, t):
                    def emit():
                        q0 = max(0, 128 * (t - 4 * j))
                        if t == 0:
                            state[hh] = ps_av.tile([65, QCHUNK], f32,
                                                   tag="av", name="z")
                        z = state[hh]
                        nc.tensor.matmul(
                            out=z[:, q0:], lhsT=Vs[:, t, b, hh, :],
                            rhs=E[:, t, hh, q0:],
                            start=(t == 0), stop=(t == nkt - 1))
                        if t == nkt - 1:
                            av_epilogue(j, b, hh, z, zsb)
                    return emit

                for hh in range(2):
                    for t in range(nkt):
                        items.append(mk(hh, t))
                return items

            def z_export(j):
                # zsb [p, b, q] -> zsend [dest=(b,qq), p, q']; q = qq*128+q'
                zsb = zsb_tiles.pop(j)
                src = zsb.rearrange("p b (r q) -> p (b r) q", r=4)
                dst = zsend[j].rearrange("c p q -> p c q")
                nc.sync.dma_start(out=dst, in_=src)
                nc.gpsimd.collective_compute(
                    "AllToAll", mybir.AluOpType.bypass,
                    replica_groups=[list(range(N_CORES))],
                    ins=[zsend[j].opt()],
                    outs=[zrecv[j].opt()])

            def outproj(j):
                # zrecv [src s, p, q'] -> zx_sb [p, K=s, q']
                zx_sb = zxpool.tile([128, 8, 128], bf16, tag="zx")
                nc.sync.dma_start(
                    out=zx_sb,
                    in_=zrecv[j].rearrange("s p q -> p s q"))
                o_sb = osbpool.tile([128, D], f16, tag="osb")
                po0 = ps_pj.tile([128, 512], f32, tag="pj")
                po1 = ps_pj.tile([128, 512], f32, tag="pj")
                for kt in range(8):
                    for mc, po in ((0, po0), (1, po1)):
                        nc.tensor.matmul(
                            out=po, lhsT=zx_sb[:, kt, :],
                            rhs=wo_sb[:, kt, mc * 512:(mc + 1) * 512],
                            start=(kt == 0), stop=(kt == 7))
                for mc, po in ((0, po0), (1, po1)):
                    nc.scalar.copy(
                        out=o_sb[:, mc * 512:(mc + 1) * 512], in_=po)
                nc.sync.dma_start(out=out_ext[j], in_=o_sb)

            stages = [(j, b) for j in range(NCHUNK) for b in range(2)]

            def weave(primary, filler):
                """Emit all of `primary`, spreading `filler` between them."""
                n = max(1, len(primary))
                ratio = (len(filler) + n - 1) // n if filler else 0
                fi = 0
                for p in primary:
                    p()
                    for _ in range(ratio):
                        if fi < len(filler):
                            filler[fi]()
                            fi += 1
                while fi < len(filler):
                    filler[fi]()
                    fi += 1

            for idx, (j, b) in enumerate(stages):
                nkt = 4 * j + 4
                E_tiles[(j, b)] = epool.tile([128, 16, 2, QCHUNK], bf16,
                                             tag="E", name="E")
                sc_items = [(lambda t=t: scores_tile(j, b, t))
                            for t in range(nkt)]
                if b == 0:
                    fetch_x(j + 1, nc.sync)
                    # av(j-1,1) weaves into proj so its normalize lands on
                    # the DVE queue ahead of this chunk's rope ops.
                    av_prev = av_items(j - 1, 1) if idx >= 1 else []
                    weave(proj_chunk_items(j), av_prev)
                    if idx >= 1:
                        z_export(j - 1)
                    weave(sc_items, proj_v_items(j))
                    if idx >= 4:
                        outproj(j - 2)
                else:
                    weave(sc_items, av_items(j, 0))
                mask(j, b)
            for it in av_items(3, 1):
                it()
            z_export(3)
            outproj(2)
            outproj(3)

    nc.compile()
    return nc


def _get_built(with_qk_bias):
    key = bool(with_qk_bias)
    if key not in _BUILT:
        _BUILT[key] = _build(key)
    return _BUILT[key]


def _rope_tables():
    pos = np.arange(S, dtype=np.float64)
    dim = np.arange(DH // 2, dtype=np.float64)
    freq = ROT_BASE ** (dim / (DH / 2))
    freq = np.concatenate([freq, freq])                # [64]
    ang = pos[None, :] / freq[:, None]                 # [64, S]
    cos = np.cos(ang)
    sin = np.sin(ang)
    # sign of the rotate-half term folded into sin': rows 0..31 get -sin
    sinm = sin.copy()
    sinm[:DH // 2] *= -1.0
    cosT = np.tile(cos, (2, 1)).astype(BF)             # [128, S]
    sinT = np.tile(sinm, (2, 1)).astype(BF)
    return cosT, sinT


def kernel(x, W_Q, b_Q, W_K, b_K, W_V, b_V, W_O, b_O):
    from concourse.bass_utils import run_bass_kernel_spmd

    x = np.asarray(x)
    W_Q, W_K, W_V, W_O = (np.asarray(a) for a in (W_Q, W_K, W_V, W_O))
    b_Q, b_K, b_V, b_O = (np.asarray(a) for a in (b_Q, b_K, b_V, b_O))
    with_qk_bias = bool(np.any(b_Q) or np.any(b_K))
    nc = _get_built(with_qk_bias)

    cosT, sinT = _rope_tables()

    def wtile(w):            # [1024, C] -> [128, 8, C]
        c = w.shape[1]
        return np.ascontiguousarray(
            w.reshape(8, 128, c).transpose(1, 0, 2)).astype(BF)

    # full output projection, head-major rows: [1024, 1024] -> [128, 8, 1024]
    wo_full = np.ascontiguousarray(
        W_O.reshape(NH * DH, D).reshape(8, 128, D).transpose(1, 0, 2)
    ).astype(BF)

    # chunk-major x so each chunk's DMA is one contiguous 2MB read:
    # [NCHUNK, 128, 2, 8, QCHUNK]
    xt = np.stack([x[b].T.reshape(8, 128, S).transpose(1, 0, 2)
                   for b in range(2)], axis=1)          # [128, 2, 8, S]
    xT_host = np.ascontiguousarray(
        xt.reshape(128, 2, 8, NCHUNK, QCHUNK).transpose(3, 0, 1, 2, 4)
    ).astype(BF)

    in_maps = []
    for core in range(N_CORES):
        h0 = 2 * core
        wq_h = wtile(np.concatenate([W_Q[h0], W_Q[h0 + 1]], axis=1))
        wk_h = wtile(np.concatenate([W_K[h0], W_K[h0 + 1]], axis=1))
        wv_h = wtile(np.concatenate([W_V[h0], W_V[h0 + 1]], axis=1))
        m = {
            "xT": xT_host, "wq": wq_h, "wk": wk_h, "wv": wv_h, "wo": wo_full,
            "cosT": cosT, "sinTm": sinT,
        }
        if with_qk_bias:
            m["bq"] = np.concatenate(
                [b_Q[h0], b_Q[h0 + 1]]).astype(np.float32).reshape(128, 1)
            m["bk"] = np.concatenate(
                [b_K[h0], b_K[h0 + 1]]).astype(np.float32).reshape(128, 1)
        in_maps.append(m)

    global _last_in_maps
    _last_in_maps = in_maps
    res = run_bass_kernel_spmd(nc, in_maps, list(range(N_CORES)))

    out = np.empty((2, S, D), dtype=np.float32)
    for core in range(N_CORES):
        b, r = divmod(core, 4)
        shard = res.results[core]["out_shard"].astype(np.float32)
        for j in range(NCHUNK):
            out[b, QCHUNK * j + 128 * r: QCHUNK * j + 128 * (r + 1), :] = \
                shard[j]

    # b_V shifts z by exactly b_V (softmax rows sum to 1); fold with b_O.
    corr = b_O.astype(np.float64).copy()
    if np.any(b_V):
        corr = corr + np.einsum("hd,hdm->m", b_V.astype(np.float64),
                                W_O.astype(np.float64))
    if np.any(corr):
        out = out + corr.astype(np.float32)
    return out


# revision 22
# speedup vs baseline: 1.1021x; 1.0305x over previous
"""Trainium2 Bass kernel for causal multi-head attention with NeoX RoPE.

Problem: x[2, 2048, 1024], 16 heads x d_head 64, rotary over all 64 dims,
causal softmax, output projection.  Sharding: head-parallel attention -
core c computes heads {2c, 2c+1} for BOTH batches; the per-head attention
outputs z are exchanged with one 8-core AllToAll per q-chunk (core c ends up
with all 16 heads for batch c//4, q-quarter c%4), then every core runs the
output projection with the full W_O for its 128 q rows per chunk.

Per-core dataflow (2 heads, 2 batches):
  xT [b, d, s] (bf16, host-transposed)
  Q/K projections -> qT/kT [head-pair 128, b, s] via PE; RoPE applied as
    q*cos + shuffle(q)*sin' with the rotate-half shuffle done by cross-
    partition copies and the flip sign folded into the sin table.
  V projection -> V [s, b, head, 65] with a ones column (softmax denominator
    rides the attention matmul for free).
  Scores computed transposed: S_T[k, q] = kT.T @ qT per 128-k-tile, exp on
    ScalarE (scale=1/8 folded in), causal mask via GPSIMD affine_select on
    diagonal tiles only.
  AV: zT_aug[65, q] += V_aug.T @ E_T accumulated over k tiles; row 64 is the
  softmax denominator.  Normalize via reciprocal_approx_fast (read straight
  from PSUM) + GPSIMD partition_broadcast + one tensor_tensor multiply.
  z exchange: zsb -> DRAM -> 8-core AllToAll -> z_all [1024, 128q].
  Output projection: out[q, m] = z_all.T @ W_O_full, f16 out via ScalarE.
"""

import numpy as np
import ml_dtypes

S = 2048
D = 1024
NH = 16
DH = 64
SCALE = 8.0
ROT_BASE = 10000.0
N_CORES = 8
QCHUNK = 512     # q chunk (free dim of score matmuls)
NCHUNK = S // QCHUNK
KTILE = 128
BF = ml_dtypes.bfloat16

_BUILT = {}


def _build(with_qk_bias):
    import concourse.bass as bass
    import concourse.tile as tile
    from concourse import bacc, mybir

    f32 = mybir.dt.float32
    bf16 = mybir.dt.bfloat16
    f16 = mybir.dt.float16
    AF = mybir.ActivationFunctionType
    OP = mybir.AluOpType

    nc = bacc.Bacc("TRN2", target_bir_lowering=False, debug=False,
                   num_devices=N_CORES)

    xT = nc.dram_tensor("xT", [NCHUNK, 128, 2, 8, QCHUNK], bf16,
                    kind="ExternalInput").ap()
    wq = nc.dram_tensor("wq", [128, 8, 128], bf16, kind="ExternalInput").ap()
    wk = nc.dram_tensor("wk", [128, 8, 128], bf16, kind="ExternalInput").ap()
    wv = nc.dram_tensor("wv", [128, 8, 128], bf16, kind="ExternalInput").ap()
    # full W_O for all 16 heads: [p, K, m], d = K*128 + p in head-major order
    wo = nc.dram_tensor("wo", [128, 8, D], bf16, kind="ExternalInput").ap()
    cosd = nc.dram_tensor("cosT", [128, S], bf16, kind="ExternalInput").ap()
    sind = nc.dram_tensor("sinTm", [128, S], bf16, kind="ExternalInput").ap()
    if with_qk_bias:
        bqd = nc.dram_tensor("bq", [128, 1], f32, kind="ExternalInput").ap()
        bkd = nc.dram_tensor("bk", [128, 1], f32, kind="ExternalInput").ap()

    out_ext = nc.dram_tensor("out_shard", [NCHUNK, 128, D], f16,
                             kind="ExternalOutput").ap()
    import os as _os
    _DBG = _os.environ.get("KDBG", "0") == "1"
    if _DBG:
        dbg_zs = nc.dram_tensor("dbg_zs", [NCHUNK, 8, 128, 128], bf16,
                                kind="ExternalOutput").ap()
        dbg_zr = nc.dram_tensor("dbg_zr", [NCHUNK, 8, 128, 128], bf16,
                                kind="ExternalOutput").ap()

    with tile.TileContext(nc) as tc:
        with (
            tc.tile_pool(name="consts", bufs=1) as consts,
            tc.tile_pool(name="qk", bufs=1) as qkpool,
            tc.tile_pool(name="vsb", bufs=1) as vpool,
            tc.tile_pool(name="rope", bufs=2) as rope,
            tc.tile_pool(name="xp", bufs=2) as xpool,
            tc.tile_pool(name="epool", bufs=2) as epool,
            tc.tile_pool(name="zpool", bufs=2) as zpool,
            tc.tile_pool(name="den", bufs=2) as den,
            tc.tile_pool(name="zx", bufs=2) as zxpool,
            tc.tile_pool(name="osb", bufs=2) as osbpool,
            tc.tile_pool(name="ps_sc", bufs=2, space="PSUM") as ps_sc,
            tc.tile_pool(name="ps_av", bufs=2, space="PSUM") as ps_av,
            tc.tile_pool(name="ps_pj", bufs=2, space="PSUM") as ps_pj,
            tc.tile_pool(name="dram", bufs=1, space="DRAM") as dram,
        ):
            # ---- input DMAs: few big transfers, priority-ordered ----
            wq_sb = consts.tile([128, 8, 128], bf16, tag="wq")
            nc.gpsimd.dma_start(out=wq_sb, in_=wq)
            wk_sb = consts.tile([128, 8, 128], bf16, tag="wk")
            nc.gpsimd.dma_start(out=wk_sb, in_=wk)
            wv_sb = consts.tile([128, 8, 128], bf16, tag="wv")
            nc.gpsimd.dma_start(out=wv_sb, in_=wv)
            wo_sb = consts.tile([128, 8, D], bf16, tag="wo")
            nc.gpsimd.dma_start(out=wo_sb, in_=wo)
            if with_qk_bias:
                bq_sb = consts.tile([128, 1], f32, tag="bq")
                nc.sync.dma_start(out=bq_sb, in_=bqd)
                bk_sb = consts.tile([128, 1], f32, tag="bk")
                nc.sync.dma_start(out=bk_sb, in_=bkd)

            # preload the exp table set so the first real exp is fast
            warm = consts.tile([128, 8], f32, tag="warm")
            nc.vector.memset(warm, 0.0)
            nc.scalar.activation(out=warm, in_=warm, func=AF.Exp, scale=1.0)

            # Persistent rotated Q/K: [128 (= head pair), batch, s]
            Qr = qkpool.tile([128, 2, S], bf16, tag="Qr")
            Kr = qkpool.tile([128, 2, S], bf16, tag="Kr")
            # V with ones column: [s-part, s-tile, batch, head, 65]
            Vs = vpool.tile([128, 16, 2, 2, 65], bf16, tag="Vs")
            nc.vector.memset(Vs[:, :, :, :, 64:65], 1.0)

            # DRAM bounce buffers for the per-chunk z AllToAll.
            # layout [8 dest, 128 p, 128 q']: dest-slab (b, qq) is the
            # collective's split dim.
            zsend = [dram.tile([8, 128, 128], bf16, tag=f"zs{j}",
                               name=f"zsend{j}") for j in range(NCHUNK)]
            zrecv = [dram.tile([8, 128, 128], bf16, tag=f"zr{j}",
                               name=f"zrecv{j}") for j in range(NCHUNK)]

            # x chunks stream through a 2-deep ring; chunk c is consumed
            # only by proj_chunk(c)/proj_v(c) at stage (c, 0).
            xc_tiles = {}

            def fetch_x(c, eng):
                if c >= NCHUNK or c in xc_tiles:
                    return
                xc = xpool.tile([128, 2, 8, QCHUNK], bf16, tag="xT",
                                name=f"xc{c}")
                eng.dma_start(out=xc, in_=xT[c])
                xc_tiles[c] = xc

            # chunk 0 split across both HWDGE queues for fastest arrival
            xc0 = xpool.tile([128, 2, 8, QCHUNK], bf16, tag="xT", name="xc0")
            nc.sync.dma_start(out=xc0[:, 0], in_=xT[0, :, 0])
            nc.scalar.dma_start(out=xc0[:, 1], in_=xT[0, :, 1])
            xc_tiles[0] = xc0
            cos_sb = consts.tile([128, S], bf16, tag="cos")
            nc.scalar.dma_start(out=cos_sb, in_=cosd)
            sin_sb = consts.tile([128, S], bf16, tag="sin")
            nc.scalar.dma_start(out=sin_sb, in_=sind)
            fetch_x(1, nc.sync)

            # ---- per-chunk projections (interleaved with attention) ----
            def proj_chunk_items(c):
                xc = xc_tiles[c]
                cs = slice(c * QCHUNK, (c + 1) * QCHUNK)
                items = []
                for b in range(2):
                    for (wsb, bias_sb, dst) in (
                        (wq_sb, "bq", Qr), (wk_sb, "bk", Kr)):
                        def emit(b=b, wsb=wsb, bias_sb=bias_sb, dst=dst):
                            pt = ps_pj.tile([128, QCHUNK], f32, tag="pj")
                            for kt in range(8):
                                nc.tensor.matmul(
                                    out=pt, lhsT=wsb[:, kt, :],
                                    rhs=xc[:, b, kt, :],
                                    start=(kt == 0), stop=(kt == 7))
                            if with_qk_bias:
                                bsb = bq_sb if bias_sb == "bq" else bk_sb
                                nc.vector.tensor_scalar_add(
                                    out=pt, in0=pt, scalar1=bsb[:, 0:1])
                            q_sb = rope.tile([128, QCHUNK], bf16, tag="ropeA")
                            nc.vector.tensor_copy(out=q_sb, in_=pt)
                            qf = rope.tile([128, QCHUNK], bf16, tag="ropeB")
                            for blk in range(4):
                                srcp = (blk ^ 1) * 32
                                nc.vector.tensor_copy(
                                    out=qf[blk * 32:blk * 32 + 32, :],
                                    in_=q_sb[srcp:srcp + 32, :])
                            # q_rot = q*cos + flip(q)*sin' (sign in sin')
                            qc = rope.tile([128, QCHUNK], bf16, tag="ropeC")
                            nc.vector.tensor_tensor(
                                out=qc, in0=q_sb, in1=cos_sb[:, cs],
                                op=OP.mult)
                            qs = rope.tile([128, QCHUNK], bf16, tag="ropeD")
                            nc.vector.tensor_tensor(
                                out=qs, in0=qf, in1=sin_sb[:, cs],
                                op=OP.mult)
                            nc.vector.tensor_tensor(
                                out=dst[:, b, cs], in0=qc, in1=qs, op=OP.add)
                        items.append(emit)
                return items

            def proj_v_items(c):
                xc = xc_tiles[c]
                items = []
                for b in range(2):
                    for lst in range(4):
                        def emit(b=b, lst=lst):
                            st = 4 * c + lst
                            pt = ps_pj.tile([128, 128], f32, tag="pj")
                            for kt in range(8):
                                nc.tensor.matmul(
                                    out=pt,
                                    lhsT=xc[:, b, kt,
                                            lst * 128:(lst + 1) * 128],
                                    rhs=wv_sb[:, kt, :],
                                    start=(kt == 0), stop=(kt == 7))
                            nc.vector.tensor_copy(
                                out=Vs[:, st, b, :, 0:64],
                                in_=pt.rearrange("p (h d) -> p h d", h=2))
                        items.append(emit)
                return items

            # ---- attention ----
            E_tiles = {}
            zsb_tiles = {}

            def scores_tile(j, b, t):
                # heads 2c (partitions 0:64) and 2c+1 (64:128) run as
                # concurrent PE row-groups.
                E = E_tiles[(j, b)]
                q0 = max(0, 128 * (t - 4 * j))
                qs2 = slice(j * QCHUNK + q0, (j + 1) * QCHUNK)
                sc = ps_sc.tile([128, 2, QCHUNK], f32, tag="sc")
                for hh in range(2):
                    hs = slice(64 * hh, 64 * hh + 64)
                    nc.tensor.matmul(
                        out=sc[:, hh, q0:],
                        lhsT=Kr[hs, b, t * 128:(t + 1) * 128],
                        rhs=Qr[hs, b, qs2], start=True, stop=True)
                nc.scalar.activation(
                    out=E[:, t, :, q0:], in_=sc[:, :, q0:],
                    func=AF.Exp, scale=1.0 / SCALE)

            def mask(j, b):
                # per diagonal tile dt, only the written span [q0:] is read
                # downstream; with v = q' - q0 the causal test is v >= p.
                E = E_tiles[(j, b)]
                for dt in range(4):
                    q0 = 128 * dt
                    nc.gpsimd.affine_select(
                        out=E[:, 4 * j + dt, :, q0:],
                        in_=E[:, 4 * j + dt, :, q0:],
                        pattern=[[0, 2], [1, QCHUNK - q0]], base=0,
                        channel_multiplier=-1,
                        compare_op=OP.is_ge, fill=0.0)

            def av_epilogue(j, b, hh, z, zsb):
                hs = slice(64 * hh, 64 * hh + 64)
                # one PSUM->SBUF copy frees the accumulation bank fast;
                # the normalize chain then runs entirely from SBUF.
                zc = den.tile([65, QCHUNK], f32, tag="zc")
                nc.vector.tensor_copy(out=zc, in_=z)
                d0 = den.tile([1, QCHUNK], f32, tag="d0")
                nc.vector.tensor_copy(out=d0, in_=zc[64:65, :])
                r0 = den.tile([1, QCHUNK], f32, tag="r0")
                nc.vector.reciprocal_approx_fast(out=r0, in_=d0)
                rb = den.tile([64, QCHUNK], f32, tag="rb")
                nc.gpsimd.partition_broadcast(out_ap=rb, in_ap=r0)
                nc.vector.tensor_tensor(
                    out=zsb[hs, b, :], in0=zc[0:64, :], in1=rb,
                    op=OP.mult)

            def av_items(j, b):
                """Per-matmul emitters for the AV accumulation of (j, b);
                interleaved into the next stage's scores stream so the PE
                always has a ready matmul while exp drains sc banks."""
                nkt = 4 * j + 4
                E = E_tiles.pop((j, b))
                if b == 0:
                    zsb_tiles[j] = zpool.tile([128, 2, QCHUNK], bf16,
                                              tag="zsb", name="zsb")
                zsb = zsb_tiles[j]
                state = {}
                items = []

                def mk(hh, t):
                    def emit():
                        q0 = max(0, 128 * (t - 4 * j))
                        if t == 0:
                            state[hh] = ps_av.tile([65, QCHUNK], f32,
                                                   tag="av", name="z")
                        z = state[hh]
                        nc.tensor.matmul(
                            out=z[:, q0:], lhsT=Vs[:, t, b, hh, :],
                            rhs=E[:, t, hh, q0:],
                            start=(t == 0), stop=(t == nkt - 1))
                        if t == nkt - 1:
                            av_epilogue(j, b, hh, z, zsb)
                    return emit

                for hh in range(2):
                    for t in range(nkt):
                        items.append(mk(hh, t))
                return items

            def z_export(j):
                # zsb [p, b, q] -> zsend [dest=(b,qq), p, q']; q = qq*128+q'
                zsb = zsb_tiles.pop(j)
                src = zsb.rearrange("p b (r q) -> p (b r) q", r=4)
                dst = zsend[j].rearrange("c p q -> p c q")
                nc.sync.dma_start(out=dst, in_=src)
                nc.gpsimd.collective_compute(
                    "AllToAll", mybir.AluOpType.bypass,
                    replica_groups=[list(range(N_CORES))],
                    ins=[zsend[j].opt()],
                    outs=[zrecv[j].opt()])

            def outproj(j):
                # zrecv [src s, p, q'] -> zx_sb [p, K=s, q']
                zx_sb = zxpool.tile([128, 8, 128], bf16, tag="zx")
                nc.sync.dma_start(
                    out=zx_sb,
                    in_=zrecv[j].rearrange("s p q -> p s q"))
                o_sb = osbpool.tile([128, D], f16, tag="osb")
                for mc in range(2):
                    po = ps_pj.tile([128, 512], f32, tag="pj")
                    for kt in range(8):
                        nc.tensor.matmul(
                            out=po, lhsT=zx_sb[:, kt, :],
                            rhs=wo_sb[:, kt, mc * 512:(mc + 1) * 512],
                            start=(kt == 0), stop=(kt == 7))
                    nc.scalar.copy(
                        out=o_sb[:, mc * 512:(mc + 1) * 512], in_=po)
                nc.sync.dma_start(out=out_ext[j], in_=o_sb)

            stages = [(j, b) for j in range(NCHUNK) for b in range(2)]
            for idx, (j, b) in enumerate(stages):
                if b == 0:
                    fetch_x(j + 1, nc.sync)
                    for it in proj_chunk_items(j):
                        it()
                    for it in proj_v_items(j):
                        it()
                nkt = 4 * j + 4
                E_tiles[(j, b)] = epool.tile([128, 16, 2, QCHUNK], bf16,
                                             tag="E", name="E")
                pending = av_items(*stages[idx - 1]) if idx >= 1 else []
                ratio = (len(pending) + nkt - 1) // nkt if pending else 0
                for t in range(nkt):
                    scores_tile(j, b, t)
                    for _ in range(ratio):
                        if pending:
                            pending.pop(0)()
                while pending:
                    pending.pop(0)()
                if idx >= 1 and stages[idx - 1][1] == 1:
                    z_export(stages[idx - 1][0])
                if idx >= 4 and b == 0:
                    outproj(j - 2)
                mask(j, b)
            for it in av_items(3, 1):
                it()
            z_export(3)
            outproj(2)
            outproj(3)

    nc.compile()
    return nc


def _get_built(with_qk_bias):
    key = bool(with_qk_bias)
    if key not in _BUILT:
        _BUILT[key] = _build(key)
    return _BUILT[key]


def _rope_tables():
    pos = np.arange(S, dtype=np.float64)
    dim = np.arange(DH // 2, dtype=np.float64)
    freq = ROT_BASE ** (dim / (DH / 2))
    freq = np.concatenate([freq, freq])                # [64]
    ang = pos[None, :] / freq[:, None]                 # [64, S]
    cos = np.cos(ang)
    sin = np.sin(ang)
    # sign of the rotate-half term folded into sin': rows 0..31 get -sin
    sinm = sin.copy()
    sinm[:DH // 2] *= -1.0
    cosT = np.tile(cos, (2, 1)).astype(BF)             # [128, S]
    sinT = np.tile(sinm, (2, 1)).astype(BF)
    return cosT, sinT


def kernel(x, W_Q, b_Q, W_K, b_K, W_V, b_V, W_O, b_O):
    from concourse.bass_utils import run_bass_kernel_spmd

    x = np.asarray(x)
    W_Q, W_K, W_V, W_O = (np.asarray(a) for a in (W_Q, W_K, W_V, W_O))
    b_Q, b_K, b_V, b_O = (np.asarray(a) for a in (b_Q, b_K, b_V, b_O))
    with_qk_bias = bool(np.any(b_Q) or np.any(b_K))
    nc = _get_built(with_qk_bias)

    cosT, sinT = _rope_tables()

    def wtile(w):            # [1024, C] -> [128, 8, C]
        c = w.shape[1]
        return np.ascontiguousarray(
            w.reshape(8, 128, c).transpose(1, 0, 2)).astype(BF)

    # full output projection, head-major rows: [1024, 1024] -> [128, 8, 1024]
    wo_full = np.ascontiguousarray(
        W_O.reshape(NH * DH, D).reshape(8, 128, D).transpose(1, 0, 2)
    ).astype(BF)

    # chunk-major x so each chunk's DMA is one contiguous 2MB read:
    # [NCHUNK, 128, 2, 8, QCHUNK]
    xt = np.stack([x[b].T.reshape(8, 128, S).transpose(1, 0, 2)
                   for b in range(2)], axis=1)          # [128, 2, 8, S]
    xT_host = np.ascontiguousarray(
        xt.reshape(128, 2, 8, NCHUNK, QCHUNK).transpose(3, 0, 1, 2, 4)
    ).astype(BF)

    in_maps = []
    for core in range(N_CORES):
        h0 = 2 * core
        wq_h = wtile(np.concatenate([W_Q[h0], W_Q[h0 + 1]], axis=1))
        wk_h = wtile(np.concatenate([W_K[h0], W_K[h0 + 1]], axis=1))
        wv_h = wtile(np.concatenate([W_V[h0], W_V[h0 + 1]], axis=1))
        m = {
            "xT": xT_host, "wq": wq_h, "wk": wk_h, "wv": wv_h, "wo": wo_full,
            "cosT": cosT, "sinTm": sinT,
        }
        if with_qk_bias:
            m["bq"] = np.concatenate(
                [b_Q[h0], b_Q[h0 + 1]]).astype(np.float32).reshape(128, 1)
            m["bk"] = np.concatenate(
                [b_K[h0], b_K[h0 + 1]]).astype(np.float32).reshape(128, 1)
        in_maps.append(m)

    global _last_in_maps
    _last_in_maps = in_maps
    res = run_bass_kernel_spmd(nc, in_maps, list(range(N_CORES)))

    out = np.empty((2, S, D), dtype=np.float32)
    for core in range(N_CORES):
        b, r = divmod(core, 4)
        shard = res.results[core]["out_shard"].astype(np.float32)
        for j in range(NCHUNK):
            out[b, QCHUNK * j + 128 * r: QCHUNK * j + 128 * (r + 1), :] = \
                shard[j]

    # b_V shifts z by exactly b_V (softmax rows sum to 1); fold with b_O.
    corr = b_O.astype(np.float64).copy()
    if np.any(b_V):
        corr = corr + np.einsum("hd,hdm->m", b_V.astype(np.float64),
                                W_O.astype(np.float64))
    if np.any(corr):
        out = out + corr.astype(np.float32)
    return out
